# revision 14
# baseline (speedup 1.0000x reference)
"""CapsuleNet forward kernel for 8 Trainium2 NeuronCores (pure data parallel).

Host side: im2col + weight-layout prep in numpy; batch 512 sharded 64/core.
Device side (per core):
  conv0:   20x20x256 stem as one K=81 matmul per image (fp32r).
  primcaps: 9x9/s2 conv as 81 tap-accumulated K=128 matmuls into PSUM.
  capsule: DMA reshuffle (channel-major -> capsule-major), squash, x_hat,
           3 dynamic-routing iterations (softmax over 10 out-caps on DVE/ACT,
           1152-capsule sums via ones-matmul on PE), |squash(s)| output.
"""

import sys

if "/opt/trn_rl_repo" not in sys.path:
    sys.path.insert(0, "/opt/trn_rl_repo")

from contextlib import ExitStack

import ml_dtypes
import numpy as np

import concourse.bacc as bacc
import concourse.bass as bass
import concourse.tile as tile
from concourse import mybir

F32 = mybir.dt.float32
F32R = mybir.dt.float32r
BF16 = mybir.dt.bfloat16
AF = mybir.ActivationFunctionType
OP = mybir.AluOpType

N_CORES = 8
B_FULL = 512
B_CORE = B_FULL // N_CORES


def _passes(B):
    if B == 64:
        return [28, 28, 8]
    # generic fallback: chunks of <=28, last chunk small
    out = []
    rem = B
    while rem > 0:
        p = min(28, rem)
        out.append(p)
        rem -= p
    return out


def _bgroups(P):
    """Split a pass of P images into matmul column groups.
    N = 36*g must be <= 512 (g<=14); keep >=8 (N>=256) for full-rate fp32r."""
    if P <= 14:
        return [P]
    if P % 2 == 0 and P // 2 <= 14:
        return [P // 2, P // 2]
    g = []
    rem = P
    while rem > 14:
        g.append(14)
        rem -= 14
    g.append(rem)
    return g


def build(B=B_CORE, passes=None):
    """Build the Bass module for one core processing B images."""
    if passes is None:
        passes = _passes(B)
    assert sum(passes) == B

    nc = bacc.Bacc("TRN2")

    # ---- DRAM I/O ----
    xcols_d = nc.dram_tensor("xcols", [B, 81, 400], BF16, kind="ExternalInput")
    c0wT_d = nc.dram_tensor("c0wT", [81, 256], BF16, kind="ExternalInput")
    c0b_d = nc.dram_tensor("c0b2", [128, 2], F32, kind="ExternalInput")
    pb_d = nc.dram_tensor("pb2", [128, 2], F32, kind="ExternalInput")
    # [tap, kt(ci//128), ci%128, co]
    wT_d = nc.dram_tensor("wT", [81, 2, 128, 256], BF16, kind="ExternalInput")
    # [m(=i//9), j(=i%9), o, k] replicated capsule weights
    dwr_d = nc.dram_tensor("dwr", [128, 9, 10, 8], F32, kind="ExternalInput")
    out_d = nc.dram_tensor("out", [B, 10], F32, kind="ExternalOutput")

    with ExitStack() as ctx:
        tc = ctx.enter_context(tile.TileContext(nc))

        consts = ctx.enter_context(tc.tile_pool(name="consts", bufs=1))
        xcp = ctx.enter_context(tc.tile_pool(name="xcp", bufs=4))
        wtp = ctx.enter_context(tc.tile_pool(name="wtp", bufs=8))
        yp = ctx.enter_context(tc.tile_pool(name="yp", bufs=1))
        s2p = ctx.enter_context(tc.tile_pool(name="s2p", bufs=2))
        up = ctx.enter_context(tc.tile_pool(name="up", bufs=1))
        xhp = ctx.enter_context(tc.tile_pool(name="xhp", bufs=1))
        tmpp = ctx.enter_context(tc.tile_pool(name="tmpp", bufs=2))
        smp = ctx.enter_context(tc.tile_pool(name="smp", bufs=1))
        pc0 = ctx.enter_context(tc.tile_pool(name="pc0", bufs=4, space="PSUM"))
        ppr = ctx.enter_context(tc.tile_pool(name="ppr", bufs=4, space="PSUM"))

        # ---- constants into SBUF ----
        c0wT_t = consts.tile([81, 256], BF16)
        nc.sync.dma_start(out=c0wT_t, in_=c0wT_d[:, :])
        c0b_t = consts.tile([128, 2], F32)
        nc.sync.dma_start(out=c0b_t, in_=c0b_d[:, :])
        pb_t = consts.tile([128, 2], F32)
        nc.sync.dma_start(out=pb_t, in_=pb_d[:, :])
        dwr_t = consts.tile([128, 9, 10, 8], F32)
        nc.sync.dma_start(out=dwr_t, in_=dwr_d[:, :, :, :])
        ones_t = consts.tile([128, 1], F32)
        nc.scalar.memzero(ones_t)
        nc.scalar.add(ones_t, ones_t, 1.0)
        ones_r = consts.tile([1, 128], F32)
        nc.vector.memset(ones_r, 1.0)
        zero_t = consts.tile([128, 1], F32)
        nc.vector.memset(zero_t, 0.0)

        b0 = 0
        for P in passes:
            bgs = _bgroups(P)

            # ================= conv0 stem =================
            # y[p, kt, b, h, w] = relu(conv0)[c=kt*128+p, b0+b, h, w]
            y_t = yp.tile([128, 2, P, 20, 20], BF16, tag="y")
            for j in range(P):
                xc_t = xcp.tile([81, 400], BF16, tag="xc")
                nc.sync.dma_start(out=xc_t, in_=xcols_d[b0 + j, :, :])
                for mt in range(2):
                    ps = pc0.tile([128, 400], F32, tag="pc0")
                    nc.tensor.matmul(
                        out=ps[:, :],
                        lhsT=c0wT_t[:, mt * 128 : (mt + 1) * 128],
                        rhs=xc_t[:, :],
                        start=True,
                        stop=True,
                    )
                    dst = y_t[:, mt, j].rearrange("p h w -> p (h w)")
                    if mt == 0:
                        nc.scalar.activation(
                            out=dst, in_=ps[:, :], func=AF.Relu,
                            bias=c0b_t[:, 0:1], scale=1.0,
                        )
                    else:
                        # (psum + bias) max 0   on DVE
                        nc.vector.scalar_tensor_tensor(
                            out=dst, in0=ps[:, :], scalar=c0b_t[:, 1:2],
                            in1=zero_t.broadcast_to([128, 400]),
                            op0=OP.add, op1=OP.max,
                        )

            # ================= primary caps conv =================
            # psum[mt][bg] accumulates over 81 taps x 2 kt
            pr_ps = [[ppr.tile([128, 36 * g], F32, tag="ppr",
                               name=f"ppr_{mt}_{gi}")
                      for gi, g in enumerate(bgs)]
                     for mt in range(2)]
            for t in range(81):
                kh, kw = t // 9, t % 9
                wt_t = wtp.tile([128, 2, 256], BF16, tag="wt")
                nc.sync.dma_start(out=wt_t, in_=wT_d[t].transpose([1, 0, 2]))
                for kt in range(2):
                    for mt in range(2):
                        gb = 0
                        for gi, g in enumerate(bgs):
                            rhs = y_t[:, kt, gb : gb + g,
                                      kh : kh + 12 : 2, kw : kw + 12 : 2]
                            nc.tensor.matmul(
                                out=pr_ps[mt][gi][:, :],
                                lhsT=wt_t[:, kt, mt * 128 : (mt + 1) * 128],
                                rhs=rhs,
                                start=(t == 0 and kt == 0),
                                stop=(t == 80 and kt == 1),
                            )
                            gb += g

            # ================= stage2: bias + s-major layout =================
            # stage2[mt][p, s, b] = prim[c=mt*128+p, b, s] + pb[c]
            s2_ts = []
            for mt in range(2):
                s2_t = s2p.tile([128, 36, P], F32, tag="s2")
                gb = 0
                for gi, g in enumerate(bgs):
                    src = pr_ps[mt][gi][:, :].rearrange("p (b s) -> p b s", s=36)
                    dst = s2_t[:, :, gb : gb + g].transpose([0, 2, 1])
                    nc.scalar.activation(
                        out=dst, in_=src, func=AF.Identity,
                        bias=pb_t[:, mt : mt + 1], scale=1.0,
                    )
                    gb += g
                s2_ts.append(s2_t)

            # ================= reshuffle to capsule layout =================
            # u[m, j, k, b] = prim_flat[b, f] , f = 72*m + 8*j + k
            # source channel c = 2m + d (d = (8j+k)//36), s = (8j+k) % 36.
            # Host permutes co so stage2[mt][d*64 + (m % 64)] == channel 2m+d
            # for m in the mt-th block of 64 pairs -> contiguous partitions.
            u_t = up.tile([128, 9, 8, P], F32, tag="u")
            for j in range(9):
                r0 = 8 * j
                pieces = []
                if r0 <= 35:
                    nk = min(8, 36 - r0)
                    pieces.append((0, 0, nk))
                    if nk < 8:
                        pieces.append((nk, 1, 8 - nk))
                else:
                    pieces.append((0, 1, 8))
                for (k0, d, nk) in pieces:
                    s0 = 8 * j + k0 - 36 * d
                    for half in range(2):
                        src = s2_ts[half][d * 64 : d * 64 + 64, s0 : s0 + nk, :]
                        dst = u_t[half * 64 : half * 64 + 64, j, k0 : k0 + nk, :]
                        nc.scalar.dma_start(out=dst, in_=src)

            # ================= squash =================
            usq = tmpp.tile([128, 9, 8, P], F32, tag="tmp")
            nc.scalar.activation(out=usq[:, :, :, :], in_=u_t[:, :, :, :],
                                 func=AF.Square)
            n2 = smp.tile([128, 9, P], F32, tag="n2")
            nc.vector.tensor_reduce(
                out=n2[:, :, :], in_=usq[:, :, :, :].transpose([0, 1, 3, 2]),
                axis=mybir.AxisListType.X, op=OP.add,
            )
            nrm = smp.tile([128, 9, P], F32, tag="nrm")
            nc.scalar.activation(out=nrm[:, :, :], in_=n2[:, :, :], func=AF.Sqrt)
            den = smp.tile([128, 9, P], F32, tag="den")
            # (n2 + 1) * nrm
            nc.vector.scalar_tensor_tensor(
                out=den[:, :, :], in0=n2[:, :, :], scalar=1.0,
                in1=nrm[:, :, :], op0=OP.add, op1=OP.mult,
            )
            nc.vector.tensor_scalar_add(den[:, :, :], den[:, :, :], 1e-8)
            rden = smp.tile([128, 9, P], F32, tag="rden")
            nc.vector.reciprocal(out=rden[:, :, :], in_=den[:, :, :])
            scl = smp.tile([128, 9, P], F32, tag="scl")
            nc.vector.tensor_mul(scl[:, :, :], n2[:, :, :], rden[:, :, :])
            # u *= scale (broadcast over k)
            nc.vector.tensor_mul(
                u_t[:, :, :, :], u_t[:, :, :, :],
                scl.unsqueeze(2).broadcast_to([128, 9, 8, P]),
            )

            # ================= x_hat =================
            # X[m, j, o, b] = sum_k dwr[m, j, o, k] * u[m, j, k, b]
            X_t = xhp.tile([128, 9, 10, P], F32, tag="X")
            for o in range(10):
                tmp = tmpp.tile([128, 9, 8, P], F32, tag="tmp")
                nc.vector.tensor_mul(
                    tmp[:, :, :, :], u_t[:, :, :, :],
                    dwr_t[:, :, o, :].unsqueeze(3).broadcast_to([128, 9, 8, P]),
                )
                nc.vector.tensor_reduce(
                    out=X_t[:, :, o, :],
                    in_=tmp[:, :, :, :].transpose([0, 1, 3, 2]),
                    axis=mybir.AxisListType.X, op=OP.add,
                )

            # ================= routing =================
            W10 = 10 * P

            def vrow(S_ps, alpha, name):
                """squash scalar: v = s^3 / ((1+s^2)(|s|+eps)), s = alpha*S."""
                ts = smp.tile([1, W10], F32, tag="v_ts")
                nc.scalar.activation(out=ts[:, :], in_=S_ps[:, :], func=AF.Copy,
                                     scale=alpha)
                s2 = smp.tile([1, W10], F32, tag="v_s2")
                nc.scalar.activation(out=s2[:, :], in_=S_ps[:, :], func=AF.Square,
                                     scale=alpha)
                ab = smp.tile([1, W10], F32, tag="v_ab")
                nc.scalar.activation(out=ab[:, :], in_=S_ps[:, :], func=AF.Abs,
                                     scale=alpha)
                dn = smp.tile([1, W10], F32, tag="v_dn")
                nc.vector.scalar_tensor_tensor(
                    out=dn[:, :], in0=s2[:, :], scalar=1.0, in1=ab[:, :],
                    op0=OP.add, op1=OP.mult,
                )
                nc.vector.tensor_scalar_add(dn[:, :], dn[:, :], 1e-8)
                rc = smp.tile([1, W10], F32, tag="v_rc")
                nc.vector.reciprocal(out=rc[:, :], in_=dn[:, :])
                nm = smp.tile([1, W10], F32, tag="v_nm")
                nc.vector.tensor_mul(nm[:, :], ts[:, :], s2[:, :])
                v = smp.tile([1, W10], F32, tag=name)
                nc.vector.tensor_mul(v[:, :], nm[:, :], rc[:, :])
                return v

            def isum(src):  # (128, 10, P) -> psum (1, 10P) via ones-matmul
                # funnel through an ACT copy so the fp32 matmul needs <=1 wait
                h = smp.tile([128, W10], F32, tag="isum_h")
                nc.scalar.copy(
                    out=h[:, :], in_=src[:, :, :].rearrange("p a b -> p (a b)")
                )
                S_ps = ppr.tile([1, W10], F32, tag="ppr")
                nc.tensor.matmul(
                    out=S_ps[:, :], lhsT=ones_t[:, :], rhs=h[:, :],
                    start=True, stop=True,
                )
                return S_ps

            # iter 0: c uniform = 1/10
            Rp = smp.tile([128, 10, P], F32, tag="Rp")
            nc.vector.tensor_reduce(
                out=Rp[:, :, :], in_=X_t[:, :, :, :].transpose([0, 2, 3, 1]),
                axis=mybir.AxisListType.X, op=OP.add,
            )
            S0 = isum(Rp)
            w_t = vrow(S0, 0.1, "w_acc")  # w = v0

            for it in (1, 2):
                # broadcast w row to all 128 partitions via K=1 ones-matmul
                w_h = smp.tile([1, W10], F32, tag="w_h")
                nc.vector.tensor_copy(out=w_h[:, :], in_=w_t[:, :])
                wb = ppr.tile([128, W10], F32, tag="ppr", name="wb_ps")
                nc.tensor.matmul(
                    out=wb[:, :], lhsT=ones_r[:, :],
                    rhs=w_h[:, :], start=True, stop=True,
                )
                L = tmpp.tile([128, 9, 10, P], F32, tag="tmp")
                nc.vector.tensor_mul(
                    L[:, :, :, :], X_t[:, :, :, :],
                    wb.rearrange("p (o b) -> p o b", o=10)
                    .unsqueeze(1).broadcast_to([128, 9, 10, P]),
                )
                nc.scalar.activation(out=L[:, :, :, :], in_=L[:, :, :, :],
                                     func=AF.Exp)
                Z = smp.tile([128, 9, P], F32, tag="Z")
                nc.vector.tensor_reduce(
                    out=Z[:, :, :], in_=L[:, :, :, :].transpose([0, 1, 3, 2]),
                    axis=mybir.AxisListType.X, op=OP.add,
                )
                rZ = smp.tile([128, 9, P], F32, tag="rZ")
                nc.vector.reciprocal(out=rZ[:, :, :], in_=Z[:, :, :])
                T1 = tmpp.tile([128, 9, 10, P], F32, tag="tmp")
                nc.vector.tensor_mul(T1[:, :, :, :], L[:, :, :, :],
                                     X_t[:, :, :, :])
                nc.vector.tensor_mul(
                    T1[:, :, :, :], T1[:, :, :, :],
                    rZ.unsqueeze(2).broadcast_to([128, 9, 10, P]),
                )
                Sp = smp.tile([128, 10, P], F32, tag="Sp")
                nc.vector.tensor_reduce(
                    out=Sp[:, :, :], in_=T1[:, :, :, :].transpose([0, 2, 3, 1]),
                    axis=mybir.AxisListType.X, op=OP.add,
                )
                S_ps = isum(Sp)
                if it == 1:
                    v1 = vrow(S_ps, 1.0, "v1")
                    nc.vector.tensor_add(w_t[:, :], w_t[:, :], v1[:, :])
                else:
                    v2 = vrow(S_ps, 1.0, "v2")
                    fo = smp.tile([1, W10], F32, tag="fo")
                    nc.scalar.activation(out=fo[:, :], in_=v2[:, :], func=AF.Abs)
                    # (1, 10, P) -> (1, P, 10) so DRAM write is contiguous
                    fob = smp.tile([1, P, 10], F32, tag="fob")
                    nc.vector.tensor_copy(
                        out=fob[:, :, :],
                        in_=fo[:, :].rearrange("p (o b) -> p o b", o=10)
                        .transpose([0, 2, 1]),
                    )
                    nc.sync.dma_start(
                        out=out_d[b0 : b0 + P, :],
                        in_=fob[:, :, :],
                    )

            b0 += P

    nc.compile()
    return nc


# ---------------- host side ----------------

_CACHE = {}


def _prep(x, conv0_w, conv0_b, prim_w, prim_b, digit_w):
    B = x.shape[0]
    xw = np.lib.stride_tricks.sliding_window_view(x[:, 0], (9, 9), axis=(1, 2))
    # (B, 20, 20, 9, 9) -> (B, 9, 9, 20, 20) -> (B, 81, 400)
    xcols = np.ascontiguousarray(
        xw.transpose(0, 3, 4, 1, 2).reshape(B, 81, 400)
    ).astype(ml_dtypes.bfloat16)
    c0wT = np.ascontiguousarray(
        conv0_w.reshape(256, 81).T
    ).astype(ml_dtypes.bfloat16)
    c0b2 = np.ascontiguousarray(
        conv0_b.reshape(2, 128).T, dtype=np.float32
    )
    # output-channel permutation: position q = mt*128 + half*64 + ml holds
    # original channel 2*(mt*64 + ml) + half  (pair-major, even/odd split)
    q = np.arange(256)
    mt, pos = q // 128, q % 128
    half, ml = pos // 64, pos % 64
    perm = 2 * (mt * 64 + ml) + half
    pb2 = np.ascontiguousarray(
        prim_b[perm].reshape(2, 128).T, dtype=np.float32
    )
    wT = np.ascontiguousarray(
        prim_w.reshape(256, 256, 81)[perm].transpose(2, 1, 0)
        .reshape(81, 2, 128, 256)
    ).astype(ml_dtypes.bfloat16)
    dwr = np.ascontiguousarray(
        digit_w[:, :, 0, :].transpose(1, 0, 2).reshape(128, 9, 10, 8),
        dtype=np.float32,
    )
    return xcols, c0wT, c0b2, pb2, wT, dwr


def kernel(x, conv0_w, conv0_b, prim_w, prim_b, digit_w):
    from concourse.bass_utils import run_bass_kernel_spmd

    x = np.asarray(x, dtype=np.float32)
    conv0_w = np.asarray(conv0_w, dtype=np.float32)
    conv0_b = np.asarray(conv0_b, dtype=np.float32)
    prim_w = np.asarray(prim_w, dtype=np.float32)
    prim_b = np.asarray(prim_b, dtype=np.float32)
    digit_w = np.asarray(digit_w, dtype=np.float32)

    xcols, c0wT, c0b2, pb2, wT, dwr = _prep(
        x, conv0_w, conv0_b, prim_w, prim_b, digit_w
    )

    if "nc" not in _CACHE:
        _CACHE["nc"] = build(B_CORE)
    nc = _CACHE["nc"]

    in_maps = []
    for c in range(N_CORES):
        sl = slice(c * B_CORE, (c + 1) * B_CORE)
        in_maps.append(
            {
                "xcols": np.ascontiguousarray(xcols[sl]),
                "c0wT": c0wT,
                "c0b2": c0b2,
                "pb2": pb2,
                "wT": wT,
                "dwr": dwr,
            }
        )
    res = run_bass_kernel_spmd(nc, in_maps, core_ids=list(range(N_CORES)))
    out = np.concatenate([r["out"] for r in res.results], axis=0)
    return out.astype(np.float32)


if __name__ == "__main__":
    # quick smoke build
    nc = build()
    print("build ok")
